# revision 15
# baseline (speedup 1.0000x reference)
"""Trainium2 Bass kernel for nn_CurvatureLoss: softmax over 4 classes ->
3 probability maps -> fused curvature-stencil chain -> masked-mean loss.

Strategy: pure data-parallel over batch (8 samples -> 8 cores). Per core,
the [4,1024,1024] sample is processed in 9 overlapping 128-row slabs.
H-direction stencils are truncated-band 128x128 matmuls on the tensor
engine (exact for each slab's owned rows); the W-direction part of the
laplacian is folded into the same PSUM accumulation via shifted-identity
matmuls. Remaining W-direction stencils are shifted-AP vector ops on
zero-padded tiles. Intermediates are fp16 (DVE 2x perf mode, full-rate
PE matmuls); PSUM accumulation and the loss sums stay fp32. Work is
spread across DVE / ACT / GpSimd. Each slab/map emits per-partition
partials (s = sum relu(-curv), c = count nonzero) into an fp32
accumulator; the host does the final tiny masked-mean reduction.
"""
import sys

if "/opt/trn_rl_repo" not in sys.path:
    sys.path.insert(0, "/opt/trn_rl_repo")

import numpy as np

P = 128
H = W = 1024
N_CORES = 8
STARTS = [0, 122, 244, 366, 488, 610, 732, 854, 896]
OWNED = [(0, 125)] + [(3, 125)] * 7 + [(83, 128)]
NSLAB = len(STARTS)
ACC_COLS = NSLAB * 3 * 2


def _band_weights():
    """fp16 lhsT weight matrices [128, 4*128]: M1.T, M2.T, M3.T, I."""
    SyP = np.eye(P, k=1, dtype=np.float64)   # (S+ x)[h] = x[h+1]
    SyM = np.eye(P, k=-1, dtype=np.float64)  # (S- x)[h] = x[h-1]
    I = np.eye(P, dtype=np.float64)
    M1 = SyP + SyM - 4 * I                   # lap = M1 @ p + (E + W)
    M2 = SyP - SyM                           # gy = M2 @ lap
    M3 = (2 * I - SyP - SyM) @ M2            # hyy = M3 @ lap
    wts = np.concatenate([M1.T, M2.T, M3.T, I], axis=1).astype(np.float16)
    return np.ascontiguousarray(wts)         # [128, 512] fp16


def _row_masks():
    masks = np.zeros((P, 3), np.float32)     # owned-row masks per slab kind
    masks[0:125, 0] = -1.0                    # first slab
    masks[3:125, 1] = -1.0                    # middle slabs
    masks[83:128, 2] = -1.0                   # last slab
    return masks


_CACHE = {}


def _build_program():
    import concourse.bacc as bacc
    import concourse.mybir as mybir
    from concourse.tile import TileContext

    f32 = mybir.dt.float32
    f16 = mybir.dt.float16
    Alu = mybir.AluOpType
    Act = mybir.ActivationFunctionType

    nc = bacc.Bacc("TRN2", target_bir_lowering=False, debug=False,
                   enable_asserts=False, num_devices=N_CORES)
    pred = nc.dram_tensor("pred", [4, H, W], f32, kind="ExternalInput").ap()
    wts = nc.dram_tensor("wts", [P, 4 * P], f16, kind="ExternalInput").ap()
    msk = nc.dram_tensor("msk", [P, 3], f32, kind="ExternalInput").ap()
    accd = nc.dram_tensor("acc", [P, ACC_COLS], f32, kind="ExternalOutput").ap()

    with TileContext(nc) as tc:
        with tc.tile_pool(name="const", bufs=1) as cpool, \
             tc.tile_pool(name="work", bufs=3) as pool, \
             tc.tile_pool(name="psum", bufs=1, space="PSUM") as ppool, \
             tc.tile_pool(name="psum2", bufs=2, space="PSUM") as ppool2, \
             nc.allow_low_precision(reason="fp16 chain validated vs reference"):
            wt = cpool.tile([P, 4 * P], f16)
            nc.sync.dma_start(out=wt[:], in_=wts)
            w1 = wt[:, 0:P]
            w2 = wt[:, P:2 * P]
            w3 = wt[:, 2 * P:3 * P]
            wI = wt[:, 3 * P:4 * P]
            mtile = cpool.tile([P, 3], f32)
            nc.sync.dma_start(out=mtile[:], in_=msk)
            acc = cpool.tile([P, ACC_COLS], f32)
            nc.vector.memset(acc[:], 0.0)
            hs = cpool.tile([P, 1], f32)          # 1/sqrt(2) bias for squares
            nc.vector.memset(hs[:], 0.7071067811865476)

            for si, st in enumerate(STARTS):
                mk = mtile[:, (0 if si == 0 else (2 if si == NSLAB - 1 else 1))
                           ][:, None]
                xt = pool.tile([P, 4, W], f32, tag="xt")
                for c in range(4):
                    nc.sync.dma_start(out=xt[:, c, :], in_=pred[c, st:st + P, :])
                # softmax over the 4 classes (no max subtraction; |x| <~ 5.5)
                ex = pool.tile([P, 4, W], f16, tag="ex")
                for c in range(4):
                    nc.scalar.activation(out=ex[:, c, :], in_=xt[:, c, :],
                                         func=Act.Exp)
                r = pool.tile([P, W], f16, tag="r")
                nc.vector.tensor_add(r, ex[:, 0, :], ex[:, 1, :])
                nc.vector.tensor_add(r, r, ex[:, 2, :])
                nc.vector.tensor_add(r, r, ex[:, 3, :])
                nc.vector.reciprocal(r, r)
                probs = pool.tile([P, 3, W + 2], f16, tag="probs")
                nc.gpsimd.memset(probs[:, :, 0:1], 0.0)
                nc.gpsimd.memset(probs[:, :, W + 1:W + 2], 0.0)
                nc.gpsimd.tensor_mul(probs[:, 0, 1:W + 1], ex[:, 1, :], r)
                t12 = pool.tile([P, W], f16, tag="t12")
                nc.vector.tensor_add(t12, ex[:, 1, :], ex[:, 2, :])
                nc.gpsimd.tensor_mul(probs[:, 1, 1:W + 1], t12, r)
                nc.gpsimd.tensor_mul(probs[:, 2, 1:W + 1], ex[:, 3, :], r)

                for m in range(3):
                    pC = probs[:, m, 1:W + 1]
                    pE = probs[:, m, 2:W + 2]
                    pW_ = probs[:, m, 0:W]
                    # lap = M1 @ pC + E + W, fully accumulated in PSUM
                    ps0 = ppool2.tile([P, W], f32, tag="ps0")
                    for hf in range(2):
                        sl = slice(hf * 512, (hf + 1) * 512)
                        nc.tensor.matmul(ps0[:, sl], lhsT=w1, rhs=pC[:, sl],
                                         start=True, stop=False)
                        nc.tensor.matmul(ps0[:, sl], lhsT=wI, rhs=pE[:, sl],
                                         start=False, stop=False)
                        nc.tensor.matmul(ps0[:, sl], lhsT=wI, rhs=pW_[:, sl],
                                         start=False, stop=True)
                    lap = pool.tile([P, W + 2], f16, tag="lap")
                    nc.gpsimd.memset(lap[:, 0:1], 0.0)
                    nc.gpsimd.memset(lap[:, W + 1:W + 2], 0.0)
                    nc.scalar.activation(out=lap[:, 1:W + 1], in_=ps0,
                                         func=Act.Copy)
                    lC = lap[:, 1:W + 1]
                    lE = lap[:, 2:W + 2]
                    lW_ = lap[:, 0:W]
                    gy = ppool.tile([P, W], f32, tag="gy")
                    hyy = ppool.tile([P, W], f32, tag="hyy")
                    for hf in range(2):
                        sl = slice(hf * 512, (hf + 1) * 512)
                        nc.tensor.matmul(gy[:, sl], lhsT=w2, rhs=lC[:, sl],
                                         start=True, stop=True)
                        nc.tensor.matmul(hyy[:, sl], lhsT=w3, rhs=lC[:, sl],
                                         start=True, stop=True)
                    gx = pool.tile([P, W + 2], f16, tag="gx")
                    nc.gpsimd.memset(gx[:, 0:1], 0.0)
                    nc.gpsimd.memset(gx[:, W + 1:W + 2], 0.0)
                    nc.vector.tensor_sub(gx[:, 1:W + 1], lE, lW_)
                    gC = gx[:, 1:W + 1]
                    gE = gx[:, 2:W + 2]
                    gW_ = gx[:, 0:W]
                    e2 = pool.tile([P, W], f16, tag="e2")
                    nc.gpsimd.tensor_add(e2, gE, gW_)
                    # hxx = 2*gx - (E+W), in place into e2
                    nc.vector.scalar_tensor_tensor(
                        out=e2, in0=gC, scalar=2.0, in1=e2,
                        op0=Alu.mult, op1=Alu.subtract)
                    hxy = pool.tile([P, W], f16, tag="hxy")
                    nc.vector.tensor_sub(hxy, gW_, gE)   # -(E-W): negation folded
                    A = pool.tile([P, W], f16, tag="A")
                    nc.scalar.activation(out=A, in_=gy, func=Act.Square,
                                         scale=0.7071067811865476, bias=hs[:])
                    C2 = pool.tile([P, W], f16, tag="C2")
                    nc.scalar.activation(out=C2, in_=gC, func=Act.Square,
                                         scale=0.7071067811865476, bias=hs[:])
                    SG = pool.tile([P, W], f16, tag="SG")
                    nc.scalar.activation(out=SG, in_=gC, func=Act.Square)
                    SG2 = pool.tile([P, W], f16, tag="SG2")
                    nc.scalar.activation(out=SG2, in_=gy, func=Act.Square)
                    T1 = pool.tile([P, W], f16, tag="T1")
                    nc.vector.tensor_mul(T1, e2, A)  # hxx * 0.5(1+gy)^2
                    G = pool.tile([P, W], f16, tag="G")
                    nc.vector.tensor_mul(G, gC, gy)
                    nc.vector.tensor_mul(G, G, hxy)  # -(gx*gy)*hxy
                    nc.vector.tensor_mul(C2, hyy, C2)  # hyy * 0.5(1+gx)^2
                    nc.gpsimd.tensor_add(T1, T1, G)
                    nc.gpsimd.tensor_add(T1, T1, C2)      # num/2
                    # D = (gx^2 + 1) + gy^2, in place into SG2
                    nc.vector.scalar_tensor_tensor(
                        out=SG2, in0=SG, scalar=1.0, in1=SG2,
                        op0=Alu.add, op1=Alu.add)
                    Rt = pool.tile([P, W], f16, tag="R")
                    nc.scalar.activation(out=Rt, in_=SG2, func=Act.Sqrt)
                    nc.vector.tensor_mul(SG2, SG2, Rt)    # D^1.5
                    nc.vector.reciprocal(SG2, SG2)        # 1 / D^1.5
                    # curv/1: negation folded into the (negative) mask
                    nc.vector.tensor_mul(T1, T1, SG2)
                    col = (si * 3 + m) * 2
                    # masked relu: Relu(curv * -mask) = relu(-curv) on owned rows
                    nc.scalar.activation(
                        out=T1, in_=T1, func=Act.Relu, scale=mk,
                        accum_out=acc[:, col:col + 1])
                    # count: out = (relu > 0); accum = add-reduce of out
                    nc.vector.tensor_scalar(
                        out=T1, in0=T1,
                        scalar1=0.0, scalar2=None, op0=Alu.is_gt, op1=Alu.add,
                        accum_out=acc[:, col + 1:col + 2])

            nc.sync.dma_start(out=accd, in_=acc[:])
    nc.compile()
    return nc


def _get_program():
    if "nc" not in _CACHE:
        _CACHE["nc"] = _build_program()
    return _CACHE["nc"]


def _run_device(pred_np):
    from concourse import bass_utils
    nc = _get_program()
    wts = _band_weights()
    msk = _row_masks()
    in_maps = [{"pred": np.ascontiguousarray(pred_np[b]), "wts": wts,
                "msk": msk}
               for b in range(N_CORES)]
    res = bass_utils.run_bass_kernel_spmd(nc, in_maps,
                                          core_ids=list(range(N_CORES)))
    return [res.results[b]["acc"] for b in range(N_CORES)]


def _host_reduce(accs):
    total = 0.0
    for b in range(N_CORES):
        a = accs[b].astype(np.float64)
        for m in range(3):
            s = a[:, [(si * 3 + m) * 2 for si in range(NSLAB)]].sum()
            c = a[:, [(si * 3 + m) * 2 + 1 for si in range(NSLAB)]].sum()
            if c > 0:
                total += s / max(c, 1.0)
    return np.float32(total)


def kernel(pred, target=None):
    assert pred.shape == (N_CORES, 4, H, W)
    accs = _run_device(np.asarray(pred, dtype=np.float32))
    return _host_reduce(accs)
